# revision 17
# baseline (speedup 1.0000x reference)
"""Trainium2 Bass kernel for the DLI (dialogue-turn ordering) loss.

Math (exact reduction of the reference):
  With 2 classes, NLL(label y) = softplus(l_{1-y} - l_y).
  u[b,j] = enc[b,j] @ (W[:D,1]-W[:D,0]),
  v[b,k] = enc[b,k] @ (W[D:,1]-W[D:,0]),
  c      = b[1]-b[0],  d[b,j,k] = u[b,j] + v[b,k] + c
  label = 1 iff k == j-1; valid pairs: k < j < len_b;  softplus(-d) = softplus(d) - d
  =>  sum_nll = sum_{valid} softplus(d) - sum_{b, 1<=j<len_b} d[b,j,j-1]
  loss = sum_nll / max(n_valid, 1)

Sharding: data-parallel over batch (64 -> 8 cores x 8). Each core ships back
the raw per-row data (u/v columns and the masked pair-sum rows) as one
[128, 19] f32 tile; the host does the final reduction, the diagonal (label-1)
terms, and the divide -- that removes the whole on-device final-reduce tail.

Engine split per core (target_regime=memory; HBM floor ~23us for 8MB):
  DMA:    enc arrives bf16 via SWDGE casting DMAs (HBM reads stay f32/8MB);
          batch 0 is split into two half-D pieces so compute starts early.
          w rows load as 8KB then broadcast down partitions SBUF->SBUF, so
          no 1MB HBM-read broadcast competes with the enc stream.
  DVE:    7 dots as fused STT(+accum); bf16 2x-mode multiplies for the other
          9 dots; su-bias adds and v-row copies
  ACT:    9 dot reductions (Copy+accum_out), per-pair Exp (bias su, reads
          PSUM, writes straight into the Ln group tile) and Ln(x+1) row-sums
  PE:     v-column transposes + [v-broadcast + triangular-NEG-mask] matmul
          pairs accumulated into PSUM (mask matmul replaces the old DVE
          tri01 multiply)
  GpSimd: ONLY SWDGE DMA triggers (no ucode ops at all -> no Q7 dge-drains)
All ACT functions (Copy/Exp/Ln) are forced into the single
natural_log_exp_and_others table so the act-table is loaded exactly once.
"""

import glob
import json
import os
import shutil
import sys
import tempfile

if "/opt/trn_rl_repo" not in sys.path:
    sys.path.insert(0, "/opt/trn_rl_repo")


def _force_combined_act_table():
    """Point walrus at an act_info.json holding only natural_log_exp_and_others
    (contains exp+ln+copy), so every ACTIVATE shares one table."""
    if os.environ.get("BASS_ACT_ROOT_JSON_PATH"):
        return
    from neuronxcc.driver.Job import Job  # type: ignore

    pwp = None
    for cand in glob.glob(os.path.join(Job.getPackageDir(), "pwp", "pwp_bin_*")):
        if os.path.exists(os.path.join(cand, "act_info.json")):
            pwp = cand
            break
    if pwp is None:
        return
    info = json.load(open(os.path.join(pwp, "act_info.json")))
    keep = [t for t in info.get("act_func_sets", [])
            if t.get("name") == "natural_log_exp_and_others"]
    if not keep:
        return
    out_dir = os.path.join(tempfile.gettempdir(), "dli_act_combined")
    os.makedirs(out_dir, exist_ok=True)
    for t in keep:
        for k in info.get("pwp_file_keys", []):
            f = t.get(k)
            src = os.path.join(pwp, f) if f else None
            if src and os.path.exists(src):
                dst = os.path.join(out_dir, f)
                if not os.path.exists(dst):
                    shutil.copy(src, dst)
    info = dict(info)
    info["act_func_sets"] = keep
    with open(os.path.join(out_dir, "act_info.json"), "w") as f:
        json.dump(info, f)
    os.environ["BASS_ACT_ROOT_JSON_PATH"] = os.path.join(out_dir, "act_info.json")


_force_combined_act_table()

from contextlib import ExitStack

import ml_dtypes
import numpy as np

import concourse.bacc as bacc
import concourse.bass as bass
import concourse.hw_specs as hw_specs
import concourse.mybir as mybir
import concourse.tile as tile

# Make bass's act-table placement agree with the trimmed act_info.json walrus
# sees: only the combined exp+ln+copy table exists, so every ACTIVATE maps to
# act_func_set_id 0 and the table is loaded exactly once.
_orig_get_act_tables = hw_specs.get_activation_tables


def _combined_act_tables(module_arch):
    tabs = _orig_get_act_tables(module_arch)
    kept = {k: v for k, v in tabs.items() if k == "natural_log_exp_and_others"}
    return kept if kept and os.environ.get("BASS_ACT_ROOT_JSON_PATH") else tabs


hw_specs.get_activation_tables = _combined_act_tables
bacc.get_activation_tables = _combined_act_tables

# Cheaper kernel teardown: drain + one all-engine barrier + sem clear. The
# walrus codegen epilogue then clears the whole sem file and barriers again on
# its own, so the stock second tile barrier only adds EVSEM latency.
from concourse.vector_clock import ScopedClock as _ScopedClock


def _cheap_drain_and_barrier(self, tick_clock, wait_clock):
    drain_inst = self.nc.sync.drain()
    wait_clock.add_sem_waits(
        drain_inst.ins, _ScopedClock({None: tick_clock.global_clock})
    )
    self.nc.all_engine_barrier()
    popped = self.nc._tile_sem_poison_stack.pop()
    assert popped is self._sem_poison
    self.nc.clear_and_free_semaphores(list(self.sems.allocated().values()))


tile.TileContext._drain_and_barrier = _cheap_drain_and_barrier

F32 = mybir.dt.float32
BF16 = mybir.dt.bfloat16
ALU = mybir.AluOpType
ACTF = mybir.ActivationFunctionType

BSZ, L, D = 64, 128, 2048
N_CORES = 8
NB = BSZ // N_CORES  # batches per core
HALF = D // 2
NEG = -30000.0  # additive mask value; exp(NEG) == 0 in f32
U_STT = {1, 5, 6, 7}  # u-dots fused on DVE (STT); late ones off ACT for the tail
V_STT = {2, 3, 4, 5}  # v-dots fused on DVE; late v split so phase B unblocks fast
LN_GROUPS = [[0, 1, 2, 3], [4, 5, 6], [7]]
NG = len(LN_GROUPS)


def build_program():
    nc = bacc.Bacc("TRN2", target_bir_lowering=False, debug=False, num_devices=1)

    enc = nc.dram_tensor("enc", [NB, L, D], F32, kind="ExternalInput").ap()
    wrow = nc.dram_tensor("wrow", [1, 2 * D], BF16, kind="ExternalInput").ap()
    auxt = nc.dram_tensor("auxt", [1, NB * L], BF16, kind="ExternalInput").ap()
    cst = nc.dram_tensor("cst", [L, 4 * L], mybir.dt.uint16, kind="ExternalInput").ap()
    out = nc.dram_tensor("out", [L, 2 * NB + NG], F32, kind="ExternalOutput").ap()

    with tile.TileContext(nc) as tc, ExitStack() as ctx:
        consts = ctx.enter_context(tc.tile_pool(name="consts", bufs=1))
        accs = ctx.enter_context(tc.tile_pool(name="accs", bufs=1))
        enc_pool = ctx.enter_context(tc.tile_pool(name="enc", bufs=3))
        junk_pool = ctx.enter_context(tc.tile_pool(name="junk", bufs=3))
        prod_pool = ctx.enter_context(tc.tile_pool(name="prod", bufs=4))
        rows_pool = ctx.enter_context(tc.tile_pool(name="rows", bufs=3))
        sp_pool = ctx.enter_context(tc.tile_pool(name="sp", bufs=1))
        exg_pool = ctx.enter_context(tc.tile_pool(name="exg", bufs=1))
        psum_d_pool = ctx.enter_context(tc.tile_pool(name="psd", bufs=2, space="PSUM"))
        psum_v_pool = ctx.enter_context(tc.tile_pool(name="psv", bufs=2, space="PSUM"))

        # ---- w rows: 8KB HWDGE load; broadcast down partitions ON-CHIP via
        # PE rank-1 matmuls into PSUM + ACT copies to SBUF. A DMA partition
        # broadcast costs ~30ns/descriptor x 128 descs and competes with the
        # enc stream; the PE+ACT route fills otherwise-idle early cycles ----
        wrow_sb = consts.tile([1, 2 * D], BF16)
        nc.sync.dma_start(wrow_sb[:], wrow[:], single_packet=True)
        auxt_sb = consts.tile([1, NB * L], BF16)
        nc.sync.dma_start(auxt_sb[:], auxt[:], single_packet=True)
        # cst u16 columns: [0:L] ident bf16 | [L:2L] tri_neg bf16 |
        # [2L:4L] ident f32 (2 u16 cols per f32 col)
        cst_sb = consts.tile([L, 4 * L], mybir.dt.uint16)
        nc.sync.dma_start(cst_sb[:], cst[:])
        ident_b = cst_sb[:, 0:L].bitcast(BF16)
        trineg_b = cst_sb[:, L : 2 * L].bitcast(BF16)
        ident_f = cst_sb[:, 2 * L : 4 * L].bitcast(F32)

        # ---- enc loads: SWDGE casting DMAs; batch 0 in two half-D pieces ----
        enc_tiles = {}
        for b in range(NB):
            t = enc_pool.tile([L, D], BF16, tag=f"enc{b % 3}", name=f"encb{b}")
            if b == 0:
                nc.gpsimd.dma_start(t[:, 0:HALF], enc[0, :, 0:HALF])
                nc.gpsimd.dma_start(t[:, HALF:D], enc[0, :, HALF:D])
            else:
                nc.gpsimd.dma_start(t[:], enc[b])
            enc_tiles[b] = t

        # ---- tiny consts built on DVE (gpsimd stays ucode-free) ----
        ones_row = consts.tile([1, L], BF16)
        nc.vector.memset(ones_row[:], 1.0)

        # ---- w broadcast: psum_w[j, f] = ones[j] * wrow[f], ACT-copy to SBUF
        psw_pool = ctx.enter_context(tc.tile_pool(name="psw", bufs=2, space="PSUM"))
        wv_b = consts.tile([L, D], BF16)
        wu_b = consts.tile([L, D], BF16)
        BANK = 512
        for wi, w_tile in ((1, wv_b), (0, wu_b)):
            for h in range(2):
                psw = psw_pool.tile([L, HALF], F32, tag="psw")
                for c in range(HALF // BANK):
                    lo = wi * D + h * HALF + c * BANK
                    nc.tensor.matmul(psw[:, c * BANK : (c + 1) * BANK],
                                     lhsT=ones_row[:],
                                     rhs=wrow_sb[0:1, lo : lo + BANK])
                nc.scalar.activation(w_tile[:, h * HALF : (h + 1) * HALF],
                                     psw[:], ACTF.Copy)

        O = accs.tile([L, 2 * NB + NG], F32)  # u cols 0..7 | v cols 8..15 | RS
        exg_tiles = [exg_pool.tile([L, len(g) * L], F32, tag=f"exg{i}", name=f"exg{i}")
                     for i, g in enumerate(LN_GROUPS)]
        grp_of = {b: (g, q) for g, grp in enumerate(LN_GROUPS) for q, b in enumerate(grp)}

        def dot_stt(enc_ap, w_tile, acc_col):
            junk = junk_pool.tile([L, D], BF16)
            nc.vector.scalar_tensor_tensor(
                out=junk[:], in0=enc_ap, scalar=1.0, op0=ALU.mult,
                in1=w_tile[:], op1=ALU.mult, accum_out=acc_col,
            )

        def dot_split(b, w_tile, acc_col):
            prod = prod_pool.tile([L, D], BF16)
            if b == 0:
                nc.vector.tensor_mul(prod[:, 0:HALF], enc_tiles[0][:, 0:HALF],
                                     w_tile[:, 0:HALF])
                nc.vector.tensor_mul(prod[:, HALF:D], enc_tiles[0][:, HALF:D],
                                     w_tile[:, HALF:D])
            else:
                nc.vector.tensor_mul(prod[:], enc_tiles[b][:], w_tile[:])
            junk = junk_pool.tile([L, D], BF16, tag="junk_act")
            nc.scalar.activation(junk[:], prod[:], ACTF.Copy, accum_out=acc_col)

        def phase_b(b):
            v_col = O[:, NB + b : NB + b + 1]
            psum_v = psum_v_pool.tile([1, L], F32)
            nc.tensor.matmul(psum_v[:], lhsT=v_col, rhs=ident_f, is_transpose=True)
            v_row = rows_pool.tile([1, L], BF16, tag="vrow")
            nc.vector.tensor_copy(v_row[:], psum_v[:])
            # psum_d[j,k] = v[k] + rmM[j] + triNEG[j,k]; exp bias adds u[j]
            psum_d = psum_d_pool.tile([L, L], F32)
            nc.tensor.matmul(psum_d[:], lhsT=ones_row[:], rhs=v_row[:],
                             start=True, stop=False)
            nc.tensor.matmul(psum_d[:], lhsT=auxt_sb[0:1, b * L : (b + 1) * L], rhs=ones_row[:],
                             start=False, stop=False)
            nc.tensor.matmul(psum_d[:], lhsT=ident_b, rhs=trineg_b,
                             start=False, stop=True)
            g, q = grp_of[b]
            nc.scalar.activation(exg_tiles[g][:, q * L : (q + 1) * L], psum_d[:],
                                 ACTF.Exp, bias=O[:, b : b + 1])

        def ln_group(g):
            sp = sp_pool.tile([L, len(LN_GROUPS[g]) * L], F32, tag=f"sp{g}")
            nc.scalar.activation(sp[:], exg_tiles[g][:], ACTF.Ln, bias=1.0,
                                 accum_out=O[:, 2 * NB + g : 2 * NB + g + 1])

        # Software-pipelined emission: phase B trails the dots by PIPE
        # batches so its small DVE ops (v-row cast, su add) never head-of-line
        # block the next batch's heavy muls in the in-order DVE queue.
        PIPE = 2

        def maybe_ln(b):
            for g, grp in enumerate(LN_GROUPS):
                if grp[-1] == b:
                    ln_group(g)

        for b in range(NB):
            if b in V_STT:
                dot_stt(enc_tiles[b][:], wv_b, O[:, NB + b : NB + b + 1])
            else:
                dot_split(b, wv_b, O[:, NB + b : NB + b + 1])
            if b in U_STT:
                dot_stt(enc_tiles[b][:], wu_b, O[:, b : b + 1])
            else:
                dot_split(b, wu_b, O[:, b : b + 1])
            if b >= PIPE:
                phase_b(b - PIPE)
                maybe_ln(b - PIPE)
        for b in range(NB - PIPE, NB):
            phase_b(b)
            maybe_ln(b)

        nc.sync.dma_start(out[:], O[:])

    nc.compile()
    return nc


_NC = None


def _get_nc():
    global _NC
    if _NC is None:
        _NC = build_program()
    return _NC


def _prep(encoder_output, mask, W, b):
    """Host-side prep: shard + derived small tensors."""
    W = np.asarray(W, dtype=np.float32)
    b = np.asarray(b, dtype=np.float32).reshape(2)
    mask = np.asarray(mask)
    c = float(b[1] - b[0])
    wrow = np.concatenate([W[:D, 1] - W[:D, 0], W[D:, 1] - W[D:, 0]]).astype(
        ml_dtypes.bfloat16
    ).reshape(1, 2 * D)
    lens = mask.astype(np.int64).sum(axis=1)  # [BSZ]
    j = np.arange(L)

    # cst layout [L, 4L] u16: [0:L] ident bf16; [L:2L] tri_neg bf16
    # (tri_neg[j,k] = 0 where k<j else NEG); [2L:4L] ident f32.
    ident_b = np.eye(L, dtype=ml_dtypes.bfloat16)
    trineg_b = np.where(j[None, :] < j[:, None], 0.0, NEG).astype(ml_dtypes.bfloat16)
    ident_f = np.eye(L, dtype=np.float32)
    cst = np.concatenate(
        [
            ident_b.view(np.uint16),
            trineg_b.view(np.uint16),
            ident_f.view(np.uint16).reshape(L, 2 * L),
        ],
        axis=1,
    )

    maps = []
    for cid in range(N_CORES):
        sl = slice(cid * NB, (cid + 1) * NB)
        lc = lens[sl]  # [NB]
        # auxt[b, j] = (j < len_b ? 0 : NEG) + c, shipped transposed so each
        # batch's row-mask is a single lhsT partition for the PE matmul
        auxt = (np.where(j[None, :] < lc[:, None], 0.0, NEG) + c).astype(
            ml_dtypes.bfloat16
        ).reshape(1, NB * L)
        maps.append(
            {
                "enc": np.ascontiguousarray(encoder_output[sl], dtype=np.float32),
                "wrow": wrow,
                "auxt": np.ascontiguousarray(auxt),
                "cst": np.ascontiguousarray(cst),
            }
        )
    return maps, lens, c


def kernel(encoder_output, mask, W, b, _run_kwargs=None):
    from concourse.bass_utils import run_bass_kernel_spmd

    nc = _get_nc()
    maps, lens, c = _prep(np.asarray(encoder_output), mask, W, b)
    res = run_bass_kernel_spmd(nc, maps, core_ids=list(range(N_CORES)),
                               **(_run_kwargs or {}))
    pair_sum = np.float64(0.0)
    diag = np.float64(0.0)
    for cid, r in enumerate(res.results):
        O = np.asarray(r["out"], dtype=np.float64)  # [L, 2NB+NG]
        pair_sum += O[:, 2 * NB :].sum()
        for bi in range(NB):
            ln = int(lens[cid * NB + bi])
            u = O[:, bi]
            v = O[:, NB + bi]
            diag += u[1:ln].sum() + v[: ln - 1].sum() + (ln - 1) * c
    n_valid = int((lens * (lens - 1) // 2).sum())
    loss = (pair_sum - diag) / max(n_valid, 1)
    out = np.array(loss, dtype=np.float32)
    if _run_kwargs is not None:
        return out, res
    return out


# revision 18
# speedup vs baseline: 1.0331x; 1.0331x over previous
"""Trainium2 Bass kernel for the DLI (dialogue-turn ordering) loss.

Math (exact reduction of the reference):
  With 2 classes, NLL(label y) = softplus(l_{1-y} - l_y).
  u[b,j] = enc[b,j] @ (W[:D,1]-W[:D,0]),
  v[b,k] = enc[b,k] @ (W[D:,1]-W[D:,0]),
  c      = b[1]-b[0],  d[b,j,k] = u[b,j] + v[b,k] + c
  label = 1 iff k == j-1; valid pairs: k < j < len_b;  softplus(-d) = softplus(d) - d
  =>  sum_nll = sum_{valid} softplus(d) - sum_{b, 1<=j<len_b} d[b,j,j-1]
  loss = sum_nll / max(n_valid, 1)

Sharding: data-parallel over batch (64 -> 8 cores x 8). Each core ships back
the raw per-row data (u/v columns and the masked pair-sum rows) as one
[128, 19] f32 tile; the host does the final reduction, the diagonal (label-1)
terms, and the divide -- that removes the whole on-device final-reduce tail.

Engine split per core (target_regime=memory; HBM floor ~23us for 8MB):
  DMA:    enc arrives bf16 via SWDGE casting DMAs (HBM reads stay f32/8MB);
          batch 0 is split into two half-D pieces so compute starts early.
          w rows load as 8KB then broadcast down partitions SBUF->SBUF, so
          no 1MB HBM-read broadcast competes with the enc stream.
  DVE:    7 dots as fused STT(+accum); bf16 2x-mode multiplies for the other
          9 dots; su-bias adds and v-row copies
  ACT:    9 dot reductions (Copy+accum_out), per-pair Exp (bias su, reads
          PSUM, writes straight into the Ln group tile) and Ln(x+1) row-sums
  PE:     v-column transposes + [v-broadcast + triangular-NEG-mask] matmul
          pairs accumulated into PSUM (mask matmul replaces the old DVE
          tri01 multiply)
  GpSimd: ONLY SWDGE DMA triggers (no ucode ops at all -> no Q7 dge-drains)
All ACT functions (Copy/Exp/Ln) are forced into the single
natural_log_exp_and_others table so the act-table is loaded exactly once.
"""

import glob
import json
import os
import shutil
import sys
import tempfile

if "/opt/trn_rl_repo" not in sys.path:
    sys.path.insert(0, "/opt/trn_rl_repo")


def _force_combined_act_table():
    """Point walrus at an act_info.json holding only natural_log_exp_and_others
    (contains exp+ln+copy), so every ACTIVATE shares one table."""
    if os.environ.get("BASS_ACT_ROOT_JSON_PATH"):
        return
    from neuronxcc.driver.Job import Job  # type: ignore

    pwp = None
    for cand in glob.glob(os.path.join(Job.getPackageDir(), "pwp", "pwp_bin_*")):
        if os.path.exists(os.path.join(cand, "act_info.json")):
            pwp = cand
            break
    if pwp is None:
        return
    info = json.load(open(os.path.join(pwp, "act_info.json")))
    keep = [t for t in info.get("act_func_sets", [])
            if t.get("name") == "natural_log_exp_and_others"]
    if not keep:
        return
    out_dir = os.path.join(tempfile.gettempdir(), "dli_act_combined")
    os.makedirs(out_dir, exist_ok=True)
    for t in keep:
        for k in info.get("pwp_file_keys", []):
            f = t.get(k)
            src = os.path.join(pwp, f) if f else None
            if src and os.path.exists(src):
                dst = os.path.join(out_dir, f)
                if not os.path.exists(dst):
                    shutil.copy(src, dst)
    info = dict(info)
    info["act_func_sets"] = keep
    with open(os.path.join(out_dir, "act_info.json"), "w") as f:
        json.dump(info, f)
    os.environ["BASS_ACT_ROOT_JSON_PATH"] = os.path.join(out_dir, "act_info.json")


_force_combined_act_table()

from contextlib import ExitStack

import ml_dtypes
import numpy as np

import concourse.bacc as bacc
import concourse.bass as bass
import concourse.hw_specs as hw_specs
import concourse.mybir as mybir
import concourse.tile as tile

# Make bass's act-table placement agree with the trimmed act_info.json walrus
# sees: only the combined exp+ln+copy table exists, so every ACTIVATE maps to
# act_func_set_id 0 and the table is loaded exactly once.
_orig_get_act_tables = hw_specs.get_activation_tables


def _combined_act_tables(module_arch):
    tabs = _orig_get_act_tables(module_arch)
    kept = {k: v for k, v in tabs.items() if k == "natural_log_exp_and_others"}
    return kept if kept and os.environ.get("BASS_ACT_ROOT_JSON_PATH") else tabs


hw_specs.get_activation_tables = _combined_act_tables
bacc.get_activation_tables = _combined_act_tables

# Cheaper kernel teardown: drain + one all-engine barrier + sem clear. The
# walrus codegen epilogue then clears the whole sem file and barriers again on
# its own, so the stock second tile barrier only adds EVSEM latency.
from concourse.vector_clock import ScopedClock as _ScopedClock


def _cheap_drain_and_barrier(self, tick_clock, wait_clock):
    drain_inst = self.nc.sync.drain()
    wait_clock.add_sem_waits(
        drain_inst.ins, _ScopedClock({None: tick_clock.global_clock})
    )
    self.nc.all_engine_barrier()
    popped = self.nc._tile_sem_poison_stack.pop()
    assert popped is self._sem_poison
    self.nc.clear_and_free_semaphores(list(self.sems.allocated().values()))


tile.TileContext._drain_and_barrier = _cheap_drain_and_barrier

F32 = mybir.dt.float32
BF16 = mybir.dt.bfloat16
ALU = mybir.AluOpType
ACTF = mybir.ActivationFunctionType

BSZ, L, D = 64, 128, 2048
N_CORES = 8
NB = BSZ // N_CORES  # batches per core
HALF = D // 2
NEG = -30000.0  # additive mask value; exp(NEG) == 0 in f32
U_STT = {1, 2, 7}  # u-dots fused on DVE (STT)
V_STT = {3, 4, 5, 6}  # v-dots fused on DVE; v7 split so the tail transpose chain starts early
LN_GROUPS = [[0, 1, 2, 3], [4, 5, 6], [7]]
NG = len(LN_GROUPS)


def build_program():
    nc = bacc.Bacc("TRN2", target_bir_lowering=False, debug=False, num_devices=1)

    enc = nc.dram_tensor("enc", [NB, L, D], F32, kind="ExternalInput").ap()
    wrow = nc.dram_tensor("wrow", [1, 2 * D], BF16, kind="ExternalInput").ap()
    auxt = nc.dram_tensor("auxt", [1, NB * L], BF16, kind="ExternalInput").ap()
    cst = nc.dram_tensor("cst", [L, 4 * L], mybir.dt.uint16, kind="ExternalInput").ap()
    out = nc.dram_tensor("out", [L, 2 * NB + NG], F32, kind="ExternalOutput").ap()

    with tile.TileContext(nc) as tc, ExitStack() as ctx:
        consts = ctx.enter_context(tc.tile_pool(name="consts", bufs=1))
        accs = ctx.enter_context(tc.tile_pool(name="accs", bufs=1))
        enc_pool = ctx.enter_context(tc.tile_pool(name="enc", bufs=3))
        junk_pool = ctx.enter_context(tc.tile_pool(name="junk", bufs=3))
        prod_pool = ctx.enter_context(tc.tile_pool(name="prod", bufs=4))
        rows_pool = ctx.enter_context(tc.tile_pool(name="rows", bufs=3))
        sp_pool = ctx.enter_context(tc.tile_pool(name="sp", bufs=1))
        exg_pool = ctx.enter_context(tc.tile_pool(name="exg", bufs=1))
        psum_d_pool = ctx.enter_context(tc.tile_pool(name="psd", bufs=2, space="PSUM"))
        psum_v_pool = ctx.enter_context(tc.tile_pool(name="psv", bufs=2, space="PSUM"))

        # ---- w rows: 8KB HWDGE load; broadcast down partitions ON-CHIP via
        # PE rank-1 matmuls into PSUM + ACT copies to SBUF. A DMA partition
        # broadcast costs ~30ns/descriptor x 128 descs and competes with the
        # enc stream; the PE+ACT route fills otherwise-idle early cycles ----
        wrow_sb = consts.tile([1, 2 * D], BF16)
        nc.sync.dma_start(wrow_sb[:], wrow[:], single_packet=True)
        auxt_sb = consts.tile([1, NB * L], BF16)
        nc.sync.dma_start(auxt_sb[:], auxt[:], single_packet=True)
        # cst u16 columns: [0:L] ident bf16 | [L:2L] tri_neg bf16 |
        # [2L:4L] ident f32 (2 u16 cols per f32 col)
        cst_sb = consts.tile([L, 4 * L], mybir.dt.uint16)
        nc.sync.dma_start(cst_sb[:], cst[:])
        ident_b = cst_sb[:, 0:L].bitcast(BF16)
        trineg_b = cst_sb[:, L : 2 * L].bitcast(BF16)
        ident_f = cst_sb[:, 2 * L : 4 * L].bitcast(F32)

        # ---- enc loads: SWDGE casting DMAs; batch 0 in two half-D pieces ----
        enc_tiles = {}
        for b in range(NB):
            t = enc_pool.tile([L, D], BF16, tag=f"enc{b % 3}", name=f"encb{b}")
            if b == 0:
                nc.gpsimd.dma_start(t[:, 0:HALF], enc[0, :, 0:HALF])
                nc.gpsimd.dma_start(t[:, HALF:D], enc[0, :, HALF:D])
            else:
                nc.gpsimd.dma_start(t[:], enc[b])
            enc_tiles[b] = t

        # ---- tiny consts built on DVE (gpsimd stays ucode-free) ----
        ones_row = consts.tile([1, L], BF16)
        nc.vector.memset(ones_row[:], 1.0)

        # ---- w broadcast: psum_w[j, f] = ones[j] * wrow[f], ACT-copy to SBUF
        psw_pool = ctx.enter_context(tc.tile_pool(name="psw", bufs=2, space="PSUM"))
        wv_b = consts.tile([L, D], BF16)
        wu_b = consts.tile([L, D], BF16)
        BANK = 512
        for wi, w_tile in ((1, wv_b), (0, wu_b)):
            for h in range(2):
                psw = psw_pool.tile([L, HALF], F32, tag="psw")
                for c in range(HALF // BANK):
                    lo = wi * D + h * HALF + c * BANK
                    nc.tensor.matmul(psw[:, c * BANK : (c + 1) * BANK],
                                     lhsT=ones_row[:],
                                     rhs=wrow_sb[0:1, lo : lo + BANK])
                nc.scalar.activation(w_tile[:, h * HALF : (h + 1) * HALF],
                                     psw[:], ACTF.Copy)

        O = accs.tile([L, 2 * NB + NG], F32)  # u cols 0..7 | v cols 8..15 | RS
        exg_tiles = [exg_pool.tile([L, len(g) * L], F32, tag=f"exg{i}", name=f"exg{i}")
                     for i, g in enumerate(LN_GROUPS)]
        grp_of = {b: (g, q) for g, grp in enumerate(LN_GROUPS) for q, b in enumerate(grp)}

        def dot_stt(enc_ap, w_tile, acc_col):
            junk = junk_pool.tile([L, D], BF16)
            nc.vector.scalar_tensor_tensor(
                out=junk[:], in0=enc_ap, scalar=1.0, op0=ALU.mult,
                in1=w_tile[:], op1=ALU.mult, accum_out=acc_col,
            )

        def dot_split(b, w_tile, acc_col):
            prod = prod_pool.tile([L, D], BF16)
            if b == 0:
                nc.vector.tensor_mul(prod[:, 0:HALF], enc_tiles[0][:, 0:HALF],
                                     w_tile[:, 0:HALF])
                nc.vector.tensor_mul(prod[:, HALF:D], enc_tiles[0][:, HALF:D],
                                     w_tile[:, HALF:D])
            else:
                nc.vector.tensor_mul(prod[:], enc_tiles[b][:], w_tile[:])
            junk = junk_pool.tile([L, D], BF16, tag="junk_act")
            nc.scalar.activation(junk[:], prod[:], ACTF.Copy, accum_out=acc_col)

        def phase_b(b):
            v_col = O[:, NB + b : NB + b + 1]
            psum_v = psum_v_pool.tile([1, L], F32)
            nc.tensor.matmul(psum_v[:], lhsT=v_col, rhs=ident_f, is_transpose=True)
            v_row = rows_pool.tile([1, L], BF16, tag="vrow")
            nc.vector.tensor_copy(v_row[:], psum_v[:])
            # psum_d[j,k] = v[k] + rmM[j] + triNEG[j,k]; exp bias adds u[j]
            psum_d = psum_d_pool.tile([L, L], F32)
            nc.tensor.matmul(psum_d[:], lhsT=ones_row[:], rhs=v_row[:],
                             start=True, stop=False)
            nc.tensor.matmul(psum_d[:], lhsT=auxt_sb[0:1, b * L : (b + 1) * L], rhs=ones_row[:],
                             start=False, stop=False)
            nc.tensor.matmul(psum_d[:], lhsT=ident_b, rhs=trineg_b,
                             start=False, stop=True)
            g, q = grp_of[b]
            nc.scalar.activation(exg_tiles[g][:, q * L : (q + 1) * L], psum_d[:],
                                 ACTF.Exp, bias=O[:, b : b + 1])

        def ln_group(g):
            sp = sp_pool.tile([L, len(LN_GROUPS[g]) * L], F32, tag=f"sp{g}")
            nc.scalar.activation(sp[:], exg_tiles[g][:], ACTF.Ln, bias=1.0,
                                 accum_out=O[:, 2 * NB + g : 2 * NB + g + 1])

        # Software-pipelined emission: phase B trails the dots by PIPE
        # batches so its small DVE ops (v-row cast, su add) never head-of-line
        # block the next batch's heavy muls in the in-order DVE queue.
        PIPE = 2

        def maybe_ln(b):
            for g, grp in enumerate(LN_GROUPS):
                if grp[-1] == b:
                    ln_group(g)

        for b in range(NB):
            if b in V_STT:
                dot_stt(enc_tiles[b][:], wv_b, O[:, NB + b : NB + b + 1])
            else:
                dot_split(b, wv_b, O[:, NB + b : NB + b + 1])
            if b in U_STT:
                dot_stt(enc_tiles[b][:], wu_b, O[:, b : b + 1])
            else:
                dot_split(b, wu_b, O[:, b : b + 1])
            if b >= PIPE:
                phase_b(b - PIPE)
                maybe_ln(b - PIPE)
        for b in range(NB - PIPE, NB):
            phase_b(b)
            maybe_ln(b)

        nc.sync.dma_start(out[:], O[:])

    nc.compile()
    return nc


_NC = None


def _get_nc():
    global _NC
    if _NC is None:
        _NC = build_program()
    return _NC


def _prep(encoder_output, mask, W, b):
    """Host-side prep: shard + derived small tensors."""
    W = np.asarray(W, dtype=np.float32)
    b = np.asarray(b, dtype=np.float32).reshape(2)
    mask = np.asarray(mask)
    c = float(b[1] - b[0])
    wrow = np.concatenate([W[:D, 1] - W[:D, 0], W[D:, 1] - W[D:, 0]]).astype(
        ml_dtypes.bfloat16
    ).reshape(1, 2 * D)
    lens = mask.astype(np.int64).sum(axis=1)  # [BSZ]
    j = np.arange(L)

    # cst layout [L, 4L] u16: [0:L] ident bf16; [L:2L] tri_neg bf16
    # (tri_neg[j,k] = 0 where k<j else NEG); [2L:4L] ident f32.
    ident_b = np.eye(L, dtype=ml_dtypes.bfloat16)
    trineg_b = np.where(j[None, :] < j[:, None], 0.0, NEG).astype(ml_dtypes.bfloat16)
    ident_f = np.eye(L, dtype=np.float32)
    cst = np.concatenate(
        [
            ident_b.view(np.uint16),
            trineg_b.view(np.uint16),
            ident_f.view(np.uint16).reshape(L, 2 * L),
        ],
        axis=1,
    )

    maps = []
    for cid in range(N_CORES):
        sl = slice(cid * NB, (cid + 1) * NB)
        lc = lens[sl]  # [NB]
        # auxt[b, j] = (j < len_b ? 0 : NEG) + c, shipped transposed so each
        # batch's row-mask is a single lhsT partition for the PE matmul
        auxt = (np.where(j[None, :] < lc[:, None], 0.0, NEG) + c).astype(
            ml_dtypes.bfloat16
        ).reshape(1, NB * L)
        maps.append(
            {
                "enc": np.ascontiguousarray(encoder_output[sl], dtype=np.float32),
                "wrow": wrow,
                "auxt": np.ascontiguousarray(auxt),
                "cst": np.ascontiguousarray(cst),
            }
        )
    return maps, lens, c


def kernel(encoder_output, mask, W, b, _run_kwargs=None):
    from concourse.bass_utils import run_bass_kernel_spmd

    nc = _get_nc()
    maps, lens, c = _prep(np.asarray(encoder_output), mask, W, b)
    res = run_bass_kernel_spmd(nc, maps, core_ids=list(range(N_CORES)),
                               **(_run_kwargs or {}))
    pair_sum = np.float64(0.0)
    diag = np.float64(0.0)
    for cid, r in enumerate(res.results):
        O = np.asarray(r["out"], dtype=np.float64)  # [L, 2NB+NG]
        pair_sum += O[:, 2 * NB :].sum()
        for bi in range(NB):
            ln = int(lens[cid * NB + bi])
            u = O[:, bi]
            v = O[:, NB + bi]
            diag += u[1:ln].sum() + v[: ln - 1].sum() + (ln - 1) * c
    n_valid = int((lens * (lens - 1) // 2).sum())
    loss = (pair_sum - diag) / max(n_valid, 1)
    out = np.array(loss, dtype=np.float32)
    if _run_kwargs is not None:
        return out, res
    return out
